# revision 22
# baseline (speedup 1.0000x reference)
"""Trainium2 Bass kernel: sparse AE encoder (L1 fan-in-1 -> relu/BN -> L2 block-diag
4x4 -> relu/BN -> L3 sparse 256-nnz/TF -> BN), SPMD over 8 NeuronCores.

Sharding: gene/hidden axis across cores (BN1/BN2 local: every core holds all 256
batch rows of its features). All layers are TensorEngine matmuls with host-packed
stationaries (L1 scatter matrix, L2 block-diagonal, L3 densified W3 shard in bf16).
BN2's mean shift is dropped (it only shifts z per-TF, which BN3 cancels), so h2 is
scale-only. Layer-3 matmuls interleave with phase B per 16-tile group. Partial z is
transposed on PE, AllToAll'd (after a warmup collective that absorbs the cold ncfw
cost during the compute phases), tree-reduced in fp32 on-core, and BN3'd locally.
Each core emits a [128, 256] outT shard.
"""

import numpy as np
import ml_dtypes

import concourse.bacc as bacc
import concourse.bass as bass
import concourse.tile as tile
import concourse.mybir as mybir
from concourse import bass_utils
from concourse.masks import make_identity

N_GENES = 8192
WM = 4
HID = N_GENES * WM          # 32768
N_TF = 1024
B = 256
EPS = 1e-5

NCORES = 8
GSH = N_GENES // NCORES     # 1024 genes / core
HSH = HID // NCORES         # 4096 hidden rows / core
P = 128
NT = HSH // P               # 32 hidden tiles / core
NGT = GSH // P              # 8 gene tiles / core
GB = 16                     # stats batching group size (tiles)

BF16 = ml_dtypes.bfloat16
F32 = mybir.dt.float32
F16 = mybir.dt.float16
BF = mybir.dt.bfloat16
AF = mybir.ActivationFunctionType
OP = mybir.AluOpType

TRACE = False
LAST_RESULT = None

_cache = {}


def _build_graph():
    nc = bacc.Bacc("TRN2", target_bir_lowering=False, debug=False, num_devices=NCORES)

    xrd = nc.dram_tensor("xrd", [P, NT * B], BF, kind="ExternalInput").ap()
    w1d = nc.dram_tensor("w1d", [P, NT], F32, kind="ExternalInput").ap()
    w2d = nc.dram_tensor("w2d", [P, NT * P], BF, kind="ExternalInput").ap()
    w3d = nc.dram_tensor("w3d", [P, NT * N_TF], BF, kind="ExternalInput").ap()
    b1d = nc.dram_tensor("b1d", [P, NT], F32, kind="ExternalInput").ap()
    b2d = nc.dram_tensor("b2d", [P, NT], F32, kind="ExternalInput").ap()
    outT = nc.dram_tensor("outT", [P, B], F32, kind="ExternalOutput").ap()

    from contextlib import ExitStack
    with tile.TileContext(nc) as tc, ExitStack() as ctx:
        cpool = ctx.enter_context(tc.tile_pool(name="const", bufs=1))
        wpool = ctx.enter_context(tc.tile_pool(name="wts", bufs=1))
        apool = ctx.enter_context(tc.tile_pool(name="acts", bufs=1))
        spool = ctx.enter_context(tc.tile_pool(name="stats", bufs=1))
        ztpool = ctx.enter_context(tc.tile_pool(name="ztile", bufs=2))
        psAB = ctx.enter_context(tc.tile_pool(name="psAB", bufs=2, space="PSUM"))
        psZp = ctx.enter_context(tc.tile_pool(name="psZ", bufs=1, space="PSUM"))
        psTp = ctx.enter_context(tc.tile_pool(name="psT", bufs=2, space="PSUM"))
        dpool = ctx.enter_context(tc.tile_pool(name="dram", bufs=1, space="DRAM"))

        # ---- static loads (contiguous, few instructions) -----------------
        b1t = cpool.tile([P, NT], F32, name="b1t")
        nc.sync.dma_start(b1t[:], b1d[:])
        w1t = cpool.tile([P, NT], F32, name="w1t")
        nc.sync.dma_start(w1t[:], w1d[:])
        xrep = wpool.tile([P, NT * B], BF, name="xrep")
        XCH = 4
        xcw = NT * B // XCH
        for c in range(XCH):
            nc.sync.dma_start(xrep[:, c * xcw:(c + 1) * xcw],
                              xrd[:, c * xcw:(c + 1) * xcw])
        b2t = cpool.tile([P, NT], F32, name="b2t")
        nc.sync.dma_start(b2t[:], b2d[:])
        w2s = wpool.tile([P, NT * P], BF, name="w2s")
        nc.sync.dma_start(w2s[:], w2d[:])
        w3s = wpool.tile([P, NT * N_TF], BF, name="w3s")
        W3CH = 8
        cw = NT * N_TF // W3CH
        for c in range(W3CH):
            nc.sync.dma_start(w3s[:, c * cw:(c + 1) * cw], w3d[:, c * cw:(c + 1) * cw])

        idt = cpool.tile([P, P], F16, name="idt")
        make_identity(nc, idt[:])
        epst = cpool.tile([P, 1], F32, name="epst")
        nc.gpsimd.memset(epst[:], EPS)

        # ---- warmup collective: absorb cold ncfw/collective cost early ---
        wdum = cpool.tile([NCORES, NCORES], BF, name="wdum")
        nc.vector.memset(wdum[:], 0.0)
        dum_in = dpool.tile([NCORES, NCORES], BF, name="dum_in")
        nc.scalar.dma_start(dum_in[:], wdum[:])
        dum_out = dpool.tile([1, NCORES], BF, name="dum_out")
        nc.gpsimd.collective_compute(
            "ReduceScatter", OP.add, replica_groups=[list(range(NCORES))],
            ins=[dum_in.opt()], outs=[dum_out.opt()])

        hr = apool.tile([P, NT * B], BF, name="hr")
        h1n = apool.tile([P, NT * B], BF, name="h1n")
        h2n = apool.tile([P, NT * B], BF, name="h2n")

        # layer-3 psums, accumulated across all NT tiles (interleaved w/ phase B)
        psZ = [[psZp.tile([P, 512], F32, name=f"psZ{bh}{th}", tag=f"psZ{bh}{th}")
                for th in range(2)] for bh in range(2)]

        def emit_l3(t):
            for bh in range(2):
                for th in range(2):
                    nc.tensor.matmul(
                        psZ[bh][th][:],
                        lhsT=h2n[:, t * B + bh * P: t * B + (bh + 1) * P],
                        rhs=w3s[:, t * N_TF + th * 512: t * N_TF + (th + 1) * 512],
                        start=(t == 0), stop=(t == NT - 1), skip_group_check=True)

        def norm_params(st, g0, istd, nm, scale_only, gb=GB):
            """bn_stats 6-tuples (even/odd halves) -> istd (and -mean*istd)."""
            sv = st[:, g0 * 6:(g0 + gb) * 6].rearrange("p (t s) -> p t s", s=6)
            me, mo = sv[:, :, 1], sv[:, :, 4]
            M2e, M2o = sv[:, :, 2], sv[:, :, 5]
            dm = spool.tile([P, gb], F32, name="dm", tag="dm")
            nc.vector.scalar_tensor_tensor(dm[:], in0=me, scalar=1.0, in1=mo,
                                           op0=OP.mult, op1=OP.subtract)
            vq = spool.tile([P, gb], F32, name="vq", tag="vq")
            nc.vector.scalar_tensor_tensor(vq[:], in0=dm[:], scalar=0.25, in1=dm[:],
                                           op0=OP.mult, op1=OP.mult)
            var = spool.tile([P, gb], F32, name="var", tag="var")
            nc.vector.scalar_tensor_tensor(var[:], in0=M2e[:, :], scalar=1.0,
                                           in1=M2o[:, :], op0=OP.mult, op1=OP.add)
            nc.vector.tensor_scalar(out=var[:], in0=var[:], scalar1=1.0 / B,
                                    scalar2=None, op0=OP.mult)
            nc.vector.tensor_tensor(var[:], var[:], vq[:], op=OP.add)
            std = spool.tile([P, gb], F32, name="std", tag="std")
            nc.scalar.activation(std[:], var[:, :gb] if False else var[:], AF.Sqrt, bias=epst[:, 0:1])
            nc.vector.reciprocal(istd[:, :gb], std[:])
            if not scale_only:
                ms = spool.tile([P, gb], F32, name="ms", tag="ms")
                nc.vector.scalar_tensor_tensor(ms[:], in0=me, scalar=1.0, in1=mo,
                                               op0=OP.mult, op1=OP.add)
                nc.vector.scalar_tensor_tensor(nm[:, :gb], in0=ms[:], scalar=-0.5,
                                               in1=istd[:, :gb], op0=OP.mult, op1=OP.mult)

        def phase(lhs_s, rhs_get, btile, dst, hrbuf, scale_only, tail_hook=None,
                  act_scale=None, gb=GB):
            st = spool.tile([P, NT * 6], F32, name="st", tag="st")
            for g0 in range(0, NT, gb):
                for t in range(g0, g0 + gb):
                    hrt = hrbuf[:, t * B:(t + 1) * B]
                    if act_scale is not None:
                        nc.scalar.activation(hrt, rhs_get(t), AF.Relu,
                                             bias=btile[:, t:t + 1],
                                             scale=act_scale[:, t:t + 1])
                    else:
                        ps = psAB.tile([P, B], F32, name="psL", tag="psL")
                        nc.tensor.matmul(ps[:], lhsT=lhs_s[:, t * P:(t + 1) * P],
                                         rhs=rhs_get(t), start=True, stop=True)
                        nc.scalar.activation(hrt, ps[:], AF.Relu,
                                             bias=btile[:, t:t + 1])
                    nc.vector.bn_stats(st[:, t * 6:(t + 1) * 6], hrt)
                istd = spool.tile([P, GB], F32, name="istd", tag="istd")
                nm = spool.tile([P, GB], F32, name="nm", tag="nm")
                norm_params(st, g0, istd, nm, scale_only, gb)
                for t in range(g0, g0 + gb):
                    eng = nc.vector if t % 2 == 0 else nc.gpsimd
                    if scale_only:
                        eng.tensor_scalar(out=dst[:, t * B:(t + 1) * B],
                                          in0=hrbuf[:, t * B:(t + 1) * B],
                                          scalar1=istd[:, t - g0:t - g0 + 1],
                                          scalar2=None, op0=OP.mult)
                    else:
                        eng.tensor_scalar(out=dst[:, t * B:(t + 1) * B],
                                          in0=hrbuf[:, t * B:(t + 1) * B],
                                          scalar1=istd[:, t - g0:t - g0 + 1],
                                          scalar2=nm[:, t - g0:t - g0 + 1],
                                          op0=OP.mult, op1=OP.add)
                    if tail_hook is not None:
                        tail_hook(t)

        phase(None, lambda t: xrep[:, t * B:(t + 1) * B], b1t, h1n, hr,
              scale_only=False, act_scale=w1t)
        # phase B: h2 is scale-only (mean shift cancels in BN3); layer-3 matmuls
        # for tile t are emitted right after h2 tile t is ready.
        phase(w2s, lambda t: h1n[:, t * B:(t + 1) * B], b2t, h2n, hr,
              scale_only=True, tail_hook=emit_l3)

        # ---- drain z, transpose to z^T, AllToAll, reduce, BN3 ------------
        zpart = apool.tile([P, 2 * N_TF], F16, name="zpart")
        for bh in range(2):
            for th in range(2):
                nc.scalar.copy(
                    zpart[:, bh * N_TF + th * 512: bh * N_TF + (th + 1) * 512],
                    psZ[bh][th][:])

        zinT = dpool.tile([N_TF, B], F16, name="zinT")
        for q in range(2):  # two DMA batches of 4 transposed tiles each
            zT4 = ztpool.tile([P, 4 * B], F16, name="zT4", tag="zT4")
            for k in range(4):
                tt = q * 4 + k
                for bh in range(2):
                    pst = psTp.tile([P, P], F16, name="pst", tag="pst")
                    nc.tensor.transpose(
                        pst[:], in_=zpart[:, bh * N_TF + tt * P: bh * N_TF + (tt + 1) * P],
                        identity=idt[:])
                    nc.scalar.copy(zT4[:, k * B + bh * P: k * B + (bh + 1) * P],
                                   pst[:])
            nc.gpsimd.dma_start(
                zinT[q * 512:(q + 1) * 512, :].rearrange("(k p) b -> p k b", p=P),
                zT4[:].rearrange("p (k b) -> p k b", k=4))

        za = dpool.tile([P, B], F16, name="za")
        nc.gpsimd.collective_compute(
            "ReduceScatter", OP.add, replica_groups=[list(range(NCORES))],
            ins=[zinT.opt()], outs=[za.opt()])
        zs = ztpool.tile([P, B], F32, name="zs", tag="zs")
        nc.gpsimd.dma_start(zs[:], za[:])

        st6 = spool.tile([P, 6], F32, name="st6", tag="st6")
        nc.vector.bn_stats(st6[:], zs[:])
        mv3 = spool.tile([P, 2], F32, name="mv3", tag="mv3")
        nc.vector.bn_aggr(mv3[:], st6[:])
        std3 = spool.tile([P, 1], F32, name="std3", tag="std3")
        nc.scalar.activation(std3[:], mv3[:, 1:2], AF.Sqrt, bias=epst[:, 0:1])
        istd3 = spool.tile([P, 1], F32, name="istd3", tag="istd3")
        nc.vector.reciprocal(istd3[:], std3[:])
        nm3 = spool.tile([P, 1], F32, name="nm3", tag="nm3")
        nc.vector.scalar_tensor_tensor(nm3[:], in0=mv3[:, 0:1], scalar=-1.0,
                                       in1=istd3[:], op0=OP.mult, op1=OP.mult)
        ofin = ztpool.tile([P, B], F32, name="ofin", tag="ofin")
        nc.vector.tensor_scalar(out=ofin[:], in0=zs[:], scalar1=istd3[:],
                                scalar2=nm3[:], op0=OP.mult, op1=OP.add)
        nc.sync.dma_start(outT[:], ofin[:])

    nc.compile()
    return nc


def _pack_inputs(features, w1, b1, w2, b2, w3, b3,
                 rows1, cols1, rows2, cols2, rows3, cols3):
    """Host-side packing into per-core contiguous [128, N] tile layouts."""
    f32 = np.float32
    features = np.asarray(features, f32)
    w1 = np.asarray(w1, f32); b1 = np.asarray(b1, f32)
    w2 = np.asarray(w2, f32); b2 = np.asarray(b2, f32)
    w3 = np.asarray(w3, f32)
    rows1 = np.asarray(rows1); cols1 = np.asarray(cols1)
    rows2 = np.asarray(rows2); cols2 = np.asarray(cols2)
    rows3 = np.asarray(rows3); cols3 = np.asarray(cols3)

    w1r = np.empty(HID, f32); w1r[rows1] = w1
    c1r = np.empty(HID, np.int64); c1r[rows1] = cols1

    order2 = np.argsort(rows2, kind="stable")
    r2 = rows2[order2]; c2 = cols2[order2]; v2 = w2[order2]

    W3d = np.zeros((HID, N_TF), f32)
    np.add.at(W3d, (cols3.astype(np.int64), rows3.astype(np.int64)), w3)

    featT = np.ascontiguousarray(features.T)  # [N_GENES, B]
    in_maps = []
    for c in range(NCORES):
        hbase = c * HSH
        # xrd[p, t*B:b] = features[b, gene_of(hid row hbase + t*128 + p)]
        genes = c1r[hbase:hbase + HSH]                      # [HSH]
        xrep = featT[genes].reshape(NT, P, B).transpose(1, 0, 2).reshape(P, NT * B)

        w2t = np.zeros((NT, P, P), f32)
        for t in range(NT):
            R0 = hbase + t * P
            es = slice(WM * R0, WM * (R0 + P))
            np.add.at(w2t[t], (c2[es] - R0, r2[es] - R0), v2[es])

        w3t = W3d[hbase:hbase + HSH].reshape(NT, P, N_TF)

        in_maps.append({
            "xrd": np.ascontiguousarray(xrep).astype(BF16),
            "w1d": np.ascontiguousarray(w1r[hbase:hbase + HSH].reshape(NT, P).T),
            "w2d": np.ascontiguousarray(w2t.transpose(1, 0, 2).reshape(P, NT * P)).astype(BF16),
            "w3d": np.ascontiguousarray(w3t.transpose(1, 0, 2).reshape(P, NT * N_TF)).astype(BF16),
            "b1d": np.ascontiguousarray(b1[hbase:hbase + HSH].reshape(NT, P).T),
            "b2d": np.ascontiguousarray(b2[hbase:hbase + HSH].reshape(NT, P).T),
        })
    return in_maps


def kernel(**inputs) -> np.ndarray:
    global LAST_RESULT
    if "nc" not in _cache:
        _cache["nc"] = _build_graph()
    nc = _cache["nc"]

    in_maps = _pack_inputs(**inputs)
    # b3 is dropped: BN3 subtracts the per-TF batch mean, so a per-TF constant
    # bias cancels exactly.

    res = bass_utils.run_bass_kernel_spmd(
        nc, in_maps, core_ids=list(range(NCORES)), trace=TRACE)
    LAST_RESULT = res

    outT = np.concatenate([res.results[c]["outT"] for c in range(NCORES)], axis=0)
    return np.ascontiguousarray(outT.T.astype(np.float32))


# revision 24
# speedup vs baseline: 1.2078x; 1.2078x over previous
"""Trainium2 Bass kernel: sparse AE encoder (L1 fan-in-1 -> relu/BN -> L2 block-diag
4x4 -> relu/BN -> L3 sparse 256-nnz/TF -> BN), SPMD over 8 NeuronCores.

Sharding: gene/hidden axis across cores (BN1/BN2 local: every core holds all 256
batch rows of its features). All layers are TensorEngine matmuls with host-packed
stationaries (L1 scatter matrix, L2 block-diagonal, L3 densified W3 shard in bf16).
BN2's mean shift is dropped (it only shifts z per-TF, which BN3 cancels), so h2 is
scale-only. Layer-3 matmuls interleave with phase B per 16-tile group. Partial z is
transposed on PE, AllToAll'd (after a warmup collective that absorbs the cold ncfw
cost during the compute phases), tree-reduced in fp32 on-core, and BN3'd locally.
Each core emits a [128, 256] outT shard.
"""

import numpy as np
import ml_dtypes

import concourse.bacc as bacc
import concourse.bass as bass
import concourse.tile as tile
import concourse.mybir as mybir
from concourse import bass_utils
from concourse.masks import make_identity

N_GENES = 8192
WM = 4
HID = N_GENES * WM          # 32768
N_TF = 1024
B = 256
EPS = 1e-5

NCORES = 8
GSH = N_GENES // NCORES     # 1024 genes / core
HSH = HID // NCORES         # 4096 hidden rows / core
P = 128
NT = HSH // P               # 32 hidden tiles / core
NGT = GSH // P              # 8 gene tiles / core
GB = 16                     # stats batching group size (tiles)

BF16 = ml_dtypes.bfloat16
F32 = mybir.dt.float32
F16 = mybir.dt.float16
BF = mybir.dt.bfloat16
AF = mybir.ActivationFunctionType
OP = mybir.AluOpType

TRACE = False
LAST_RESULT = None

_cache = {}


def _build_graph():
    nc = bacc.Bacc("TRN2", target_bir_lowering=False, debug=False, num_devices=NCORES)

    xrd = nc.dram_tensor("xrd", [P, NT * B], BF, kind="ExternalInput").ap()
    w1d = nc.dram_tensor("w1d", [P, NT], F32, kind="ExternalInput").ap()
    w2d = nc.dram_tensor("w2d", [P, NT * P], BF, kind="ExternalInput").ap()
    w3d = nc.dram_tensor("w3d", [P, NT * N_TF], BF, kind="ExternalInput").ap()
    b1d = nc.dram_tensor("b1d", [P, NT], F32, kind="ExternalInput").ap()
    b2d = nc.dram_tensor("b2d", [P, NT], F32, kind="ExternalInput").ap()
    outT = nc.dram_tensor("outT", [P, B], F32, kind="ExternalOutput").ap()

    from contextlib import ExitStack
    with tile.TileContext(nc) as tc, ExitStack() as ctx:
        cpool = ctx.enter_context(tc.tile_pool(name="const", bufs=1))
        wpool = ctx.enter_context(tc.tile_pool(name="wts", bufs=1))
        apool = ctx.enter_context(tc.tile_pool(name="acts", bufs=1))
        spool = ctx.enter_context(tc.tile_pool(name="stats", bufs=1))
        ztpool = ctx.enter_context(tc.tile_pool(name="ztile", bufs=2))
        psAB = ctx.enter_context(tc.tile_pool(name="psAB", bufs=2, space="PSUM"))
        psZp = ctx.enter_context(tc.tile_pool(name="psZ", bufs=1, space="PSUM"))
        psTp = ctx.enter_context(tc.tile_pool(name="psT", bufs=2, space="PSUM"))
        dpool = ctx.enter_context(tc.tile_pool(name="dram", bufs=1, space="DRAM"))

        # ---- static loads (contiguous, few instructions) -----------------
        b1t = cpool.tile([P, NT], F32, name="b1t")
        nc.sync.dma_start(b1t[:], b1d[:])
        w1t = cpool.tile([P, NT], F32, name="w1t")
        nc.sync.dma_start(w1t[:], w1d[:])
        xrep = wpool.tile([P, NT * B], BF, name="xrep")
        XCH = 4
        xcw = NT * B // XCH
        for c in range(XCH):
            nc.sync.dma_start(xrep[:, c * xcw:(c + 1) * xcw],
                              xrd[:, c * xcw:(c + 1) * xcw])
        b2t = cpool.tile([P, NT], F32, name="b2t")
        nc.sync.dma_start(b2t[:], b2d[:])
        w2s = wpool.tile([P, NT * P], BF, name="w2s")
        nc.sync.dma_start(w2s[:], w2d[:])
        w3s = wpool.tile([P, NT * N_TF], BF, name="w3s")
        W3CH = 8
        cw = NT * N_TF // W3CH
        for c in range(W3CH):
            nc.sync.dma_start(w3s[:, c * cw:(c + 1) * cw], w3d[:, c * cw:(c + 1) * cw])

        idt = cpool.tile([P, P], F16, name="idt")
        make_identity(nc, idt[:])
        epst = cpool.tile([P, 1], F32, name="epst")
        nc.gpsimd.memset(epst[:], EPS)

        # ---- warmup collective: absorb cold ncfw/collective cost early ---
        wdum = cpool.tile([NCORES, NCORES], BF, name="wdum")
        nc.vector.memset(wdum[:], 0.0)
        dum_in = dpool.tile([NCORES, NCORES], BF, name="dum_in")
        nc.scalar.dma_start(dum_in[:], wdum[:])
        dum_out = dpool.tile([1, NCORES], BF, name="dum_out")
        nc.gpsimd.collective_compute(
            "ReduceScatter", OP.add, replica_groups=[list(range(NCORES))],
            ins=[dum_in.opt()], outs=[dum_out.opt()])

        hr = apool.tile([P, NT * B], BF, name="hr")
        h1n = apool.tile([P, NT * B], BF, name="h1n")
        h2n = apool.tile([P, NT * B], BF, name="h2n")

        # layer-3 psums, accumulated across all NT tiles (interleaved w/ phase B)
        psZ = [[psZp.tile([P, 512], F32, name=f"psZ{bh}{th}", tag=f"psZ{bh}{th}")
                for th in range(2)] for bh in range(2)]

        def emit_l3(t):
            for bh in range(2):
                for th in range(2):
                    nc.tensor.matmul(
                        psZ[bh][th][:],
                        lhsT=h2n[:, t * B + bh * P: t * B + (bh + 1) * P],
                        rhs=w3s[:, t * N_TF + th * 512: t * N_TF + (th + 1) * 512],
                        start=(t == 0), stop=(t == NT - 1), skip_group_check=True)

        def norm_params(st, g0, istd, nm, scale_only, gb=GB):
            """bn_stats 6-tuples (even/odd halves) -> istd (and -mean*istd)."""
            sv = st[:, g0 * 6:(g0 + gb) * 6].rearrange("p (t s) -> p t s", s=6)
            me, mo = sv[:, :, 1], sv[:, :, 4]
            M2e, M2o = sv[:, :, 2], sv[:, :, 5]
            dm = spool.tile([P, gb], F32, name="dm", tag="dm")
            nc.vector.scalar_tensor_tensor(dm[:], in0=me, scalar=1.0, in1=mo,
                                           op0=OP.mult, op1=OP.subtract)
            vq = spool.tile([P, gb], F32, name="vq", tag="vq")
            nc.vector.scalar_tensor_tensor(vq[:], in0=dm[:], scalar=0.25, in1=dm[:],
                                           op0=OP.mult, op1=OP.mult)
            var = spool.tile([P, gb], F32, name="var", tag="var")
            nc.vector.scalar_tensor_tensor(var[:], in0=M2e[:, :], scalar=1.0,
                                           in1=M2o[:, :], op0=OP.mult, op1=OP.add)
            nc.vector.tensor_scalar(out=var[:], in0=var[:], scalar1=1.0 / B,
                                    scalar2=None, op0=OP.mult)
            nc.vector.tensor_tensor(var[:], var[:], vq[:], op=OP.add)
            std = spool.tile([P, gb], F32, name="std", tag="std")
            nc.scalar.activation(std[:], var[:, :gb] if False else var[:], AF.Sqrt, bias=epst[:, 0:1])
            nc.vector.reciprocal(istd[:, :gb], std[:])
            if not scale_only:
                ms = spool.tile([P, gb], F32, name="ms", tag="ms")
                nc.vector.scalar_tensor_tensor(ms[:], in0=me, scalar=1.0, in1=mo,
                                               op0=OP.mult, op1=OP.add)
                nc.vector.scalar_tensor_tensor(nm[:, :gb], in0=ms[:], scalar=-0.5,
                                               in1=istd[:, :gb], op0=OP.mult, op1=OP.mult)

        def phase(lhs_s, rhs_get, btile, dst, hrbuf, scale_only, tail_hook=None,
                  act_scale=None, gb=GB):
            st = spool.tile([P, NT * 6], F32, name="st", tag="st")
            for g0 in range(0, NT, gb):
                for t in range(g0, g0 + gb):
                    hrt = hrbuf[:, t * B:(t + 1) * B]
                    if act_scale is not None:
                        nc.scalar.activation(hrt, rhs_get(t), AF.Relu,
                                             bias=btile[:, t:t + 1],
                                             scale=act_scale[:, t:t + 1])
                    else:
                        ps = psAB.tile([P, B], F32, name="psL", tag="psL")
                        nc.tensor.matmul(ps[:], lhsT=lhs_s[:, t * P:(t + 1) * P],
                                         rhs=rhs_get(t), start=True, stop=True)
                        nc.scalar.activation(hrt, ps[:], AF.Relu,
                                             bias=btile[:, t:t + 1])
                    nc.vector.bn_stats(st[:, t * 6:(t + 1) * 6], hrt)
                istd = spool.tile([P, GB], F32, name="istd", tag="istd")
                nm = spool.tile([P, GB], F32, name="nm", tag="nm")
                norm_params(st, g0, istd, nm, scale_only, gb)
                for t in range(g0, g0 + gb):
                    if scale_only:
                        nc.vector.tensor_scalar(out=dst[:, t * B:(t + 1) * B],
                                                in0=hrbuf[:, t * B:(t + 1) * B],
                                                scalar1=istd[:, t - g0:t - g0 + 1],
                                                scalar2=None, op0=OP.mult)
                    else:
                        nc.gpsimd.tensor_scalar(out=dst[:, t * B:(t + 1) * B],
                                                in0=hrbuf[:, t * B:(t + 1) * B],
                                                scalar1=istd[:, t - g0:t - g0 + 1],
                                                scalar2=nm[:, t - g0:t - g0 + 1],
                                                op0=OP.mult, op1=OP.add)
                    if tail_hook is not None:
                        tail_hook(t)

        phase(None, lambda t: xrep[:, t * B:(t + 1) * B], b1t, h1n, hr,
              scale_only=False, act_scale=w1t)
        # phase B: h2 is scale-only (mean shift cancels in BN3); layer-3 matmuls
        # for tile t are emitted right after h2 tile t is ready.
        phase(w2s, lambda t: h1n[:, t * B:(t + 1) * B], b2t, h2n, hr,
              scale_only=True, tail_hook=emit_l3, gb=8)

        # ---- drain z, transpose to z^T, AllToAll, reduce, BN3 ------------
        zpart = apool.tile([P, 2 * N_TF], F16, name="zpart")
        for bh in range(2):
            for th in range(2):
                nc.scalar.copy(
                    zpart[:, bh * N_TF + th * 512: bh * N_TF + (th + 1) * 512],
                    psZ[bh][th][:])

        zinT = dpool.tile([N_TF, B], F16, name="zinT")
        for q in range(2):  # two DMA batches of 4 transposed tiles each
            zT4 = ztpool.tile([P, 4 * B], F16, name="zT4", tag="zT4")
            for k in range(4):
                tt = q * 4 + k
                for bh in range(2):
                    pst = psTp.tile([P, P], F16, name="pst", tag="pst")
                    nc.tensor.transpose(
                        pst[:], in_=zpart[:, bh * N_TF + tt * P: bh * N_TF + (tt + 1) * P],
                        identity=idt[:])
                    nc.scalar.copy(zT4[:, k * B + bh * P: k * B + (bh + 1) * P],
                                   pst[:])
            nc.gpsimd.dma_start(
                zinT[q * 512:(q + 1) * 512, :].rearrange("(k p) b -> p k b", p=P),
                zT4[:].rearrange("p (k b) -> p k b", k=4))

        za = dpool.tile([P, B], F16, name="za")
        nc.gpsimd.collective_compute(
            "ReduceScatter", OP.add, replica_groups=[list(range(NCORES))],
            ins=[zinT.opt()], outs=[za.opt()])
        zs = ztpool.tile([P, B], F32, name="zs", tag="zs")
        nc.gpsimd.dma_start(zs[:], za[:])

        st6 = spool.tile([P, 6], F32, name="st6", tag="st6")
        nc.vector.bn_stats(st6[:], zs[:])
        mv3 = spool.tile([P, 2], F32, name="mv3", tag="mv3")
        nc.vector.bn_aggr(mv3[:], st6[:])
        std3 = spool.tile([P, 1], F32, name="std3", tag="std3")
        nc.scalar.activation(std3[:], mv3[:, 1:2], AF.Sqrt, bias=epst[:, 0:1])
        istd3 = spool.tile([P, 1], F32, name="istd3", tag="istd3")
        nc.vector.reciprocal(istd3[:], std3[:])
        nm3 = spool.tile([P, 1], F32, name="nm3", tag="nm3")
        nc.vector.scalar_tensor_tensor(nm3[:], in0=mv3[:, 0:1], scalar=-1.0,
                                       in1=istd3[:], op0=OP.mult, op1=OP.mult)
        ofin = ztpool.tile([P, B], F32, name="ofin", tag="ofin")
        nc.vector.tensor_scalar(out=ofin[:], in0=zs[:], scalar1=istd3[:],
                                scalar2=nm3[:], op0=OP.mult, op1=OP.add)
        nc.sync.dma_start(outT[:], ofin[:])

    nc.compile()
    return nc


def _pack_inputs(features, w1, b1, w2, b2, w3, b3,
                 rows1, cols1, rows2, cols2, rows3, cols3):
    """Host-side packing into per-core contiguous [128, N] tile layouts."""
    f32 = np.float32
    features = np.asarray(features, f32)
    w1 = np.asarray(w1, f32); b1 = np.asarray(b1, f32)
    w2 = np.asarray(w2, f32); b2 = np.asarray(b2, f32)
    w3 = np.asarray(w3, f32)
    rows1 = np.asarray(rows1); cols1 = np.asarray(cols1)
    rows2 = np.asarray(rows2); cols2 = np.asarray(cols2)
    rows3 = np.asarray(rows3); cols3 = np.asarray(cols3)

    w1r = np.empty(HID, f32); w1r[rows1] = w1
    c1r = np.empty(HID, np.int64); c1r[rows1] = cols1

    order2 = np.argsort(rows2, kind="stable")
    r2 = rows2[order2]; c2 = cols2[order2]; v2 = w2[order2]

    W3d = np.zeros((HID, N_TF), f32)
    np.add.at(W3d, (cols3.astype(np.int64), rows3.astype(np.int64)), w3)

    featT = np.ascontiguousarray(features.T)  # [N_GENES, B]
    in_maps = []
    for c in range(NCORES):
        hbase = c * HSH
        # xrd[p, t*B:b] = features[b, gene_of(hid row hbase + t*128 + p)]
        genes = c1r[hbase:hbase + HSH]                      # [HSH]
        xrep = featT[genes].reshape(NT, P, B).transpose(1, 0, 2).reshape(P, NT * B)

        w2t = np.zeros((NT, P, P), f32)
        for t in range(NT):
            R0 = hbase + t * P
            es = slice(WM * R0, WM * (R0 + P))
            np.add.at(w2t[t], (c2[es] - R0, r2[es] - R0), v2[es])

        w3t = W3d[hbase:hbase + HSH].reshape(NT, P, N_TF)

        in_maps.append({
            "xrd": np.ascontiguousarray(xrep).astype(BF16),
            "w1d": np.ascontiguousarray(w1r[hbase:hbase + HSH].reshape(NT, P).T),
            "w2d": np.ascontiguousarray(w2t.transpose(1, 0, 2).reshape(P, NT * P)).astype(BF16),
            "w3d": np.ascontiguousarray(w3t.transpose(1, 0, 2).reshape(P, NT * N_TF)).astype(BF16),
            "b1d": np.ascontiguousarray(b1[hbase:hbase + HSH].reshape(NT, P).T),
            "b2d": np.ascontiguousarray(b2[hbase:hbase + HSH].reshape(NT, P).T),
        })
    return in_maps


def kernel(**inputs) -> np.ndarray:
    global LAST_RESULT
    if "nc" not in _cache:
        _cache["nc"] = _build_graph()
    nc = _cache["nc"]

    in_maps = _pack_inputs(**inputs)
    # b3 is dropped: BN3 subtracts the per-TF batch mean, so a per-TF constant
    # bias cancels exactly.

    res = bass_utils.run_bass_kernel_spmd(
        nc, in_maps, core_ids=list(range(NCORES)), trace=TRACE)
    LAST_RESULT = res

    outT = np.concatenate([res.results[c]["outT"] for c in range(NCORES)], axis=0)
    return np.ascontiguousarray(outT.T.astype(np.float32))


# revision 25
# speedup vs baseline: 1.2251x; 1.0143x over previous
"""Trainium2 Bass kernel: sparse AE encoder (L1 fan-in-1 -> relu/BN -> L2 block-diag
4x4 -> relu/BN -> L3 sparse 256-nnz/TF -> BN), SPMD over 8 NeuronCores.

Sharding: gene/hidden axis across cores (BN1/BN2 local: every core holds all 256
batch rows of its features). All layers are TensorEngine matmuls with host-packed
stationaries (L1 scatter matrix, L2 block-diagonal, L3 densified W3 shard in bf16).
BN2's mean shift is dropped (it only shifts z per-TF, which BN3 cancels), so h2 is
scale-only. Layer-3 matmuls interleave with phase B per 16-tile group. Partial z is
transposed on PE, AllToAll'd (after a warmup collective that absorbs the cold ncfw
cost during the compute phases), tree-reduced in fp32 on-core, and BN3'd locally.
Each core emits a [128, 256] outT shard.
"""

import numpy as np
import ml_dtypes

import concourse.bacc as bacc
import concourse.bass as bass
import concourse.tile as tile
import concourse.mybir as mybir
from concourse import bass_utils
from concourse.masks import make_identity

N_GENES = 8192
WM = 4
HID = N_GENES * WM          # 32768
N_TF = 1024
B = 256
EPS = 1e-5

NCORES = 8
GSH = N_GENES // NCORES     # 1024 genes / core
HSH = HID // NCORES         # 4096 hidden rows / core
P = 128
NT = HSH // P               # 32 hidden tiles / core
NGT = GSH // P              # 8 gene tiles / core
GB = 16                     # stats batching group size (tiles)

BF16 = ml_dtypes.bfloat16
F32 = mybir.dt.float32
F16 = mybir.dt.float16
BF = mybir.dt.bfloat16
AF = mybir.ActivationFunctionType
OP = mybir.AluOpType

TRACE = False
LAST_RESULT = None

_cache = {}


def _build_graph():
    nc = bacc.Bacc("TRN2", target_bir_lowering=False, debug=False, num_devices=NCORES)

    xrd = nc.dram_tensor("xrd", [P, NT * B], BF, kind="ExternalInput").ap()
    w1d = nc.dram_tensor("w1d", [P, NT], F32, kind="ExternalInput").ap()
    w2d = nc.dram_tensor("w2d", [P, NT * P], BF, kind="ExternalInput").ap()
    w3d = nc.dram_tensor("w3d", [P, NT * N_TF], BF, kind="ExternalInput").ap()
    b1d = nc.dram_tensor("b1d", [P, NT], F32, kind="ExternalInput").ap()
    b2d = nc.dram_tensor("b2d", [P, NT], F32, kind="ExternalInput").ap()
    outT = nc.dram_tensor("outT", [P, B], F32, kind="ExternalOutput").ap()

    from contextlib import ExitStack
    with tile.TileContext(nc) as tc, ExitStack() as ctx:
        cpool = ctx.enter_context(tc.tile_pool(name="const", bufs=1))
        wpool = ctx.enter_context(tc.tile_pool(name="wts", bufs=1))
        apool = ctx.enter_context(tc.tile_pool(name="acts", bufs=1))
        spool = ctx.enter_context(tc.tile_pool(name="stats", bufs=1))
        ztpool = ctx.enter_context(tc.tile_pool(name="ztile", bufs=2))
        psAB = ctx.enter_context(tc.tile_pool(name="psAB", bufs=2, space="PSUM"))
        psZp = ctx.enter_context(tc.tile_pool(name="psZ", bufs=1, space="PSUM"))
        psTp = ctx.enter_context(tc.tile_pool(name="psT", bufs=2, space="PSUM"))
        dpool = ctx.enter_context(tc.tile_pool(name="dram", bufs=1, space="DRAM"))

        # ---- static loads (contiguous, few instructions) -----------------
        b1t = cpool.tile([P, NT], F32, name="b1t")
        nc.sync.dma_start(b1t[:], b1d[:])
        w1t = cpool.tile([P, NT], F32, name="w1t")
        nc.sync.dma_start(w1t[:], w1d[:])
        xrep = wpool.tile([P, NT * B], BF, name="xrep")
        XCH = 4
        xcw = NT * B // XCH
        for c in range(XCH):
            nc.sync.dma_start(xrep[:, c * xcw:(c + 1) * xcw],
                              xrd[:, c * xcw:(c + 1) * xcw])
        b2t = cpool.tile([P, NT], F32, name="b2t")
        nc.sync.dma_start(b2t[:], b2d[:])
        w2s = wpool.tile([P, NT * P], BF, name="w2s")
        nc.sync.dma_start(w2s[:], w2d[:])
        w3s = wpool.tile([P, NT * N_TF], BF, name="w3s")
        W3CH = 8
        cw = NT * N_TF // W3CH
        for c in range(W3CH):
            nc.sync.dma_start(w3s[:, c * cw:(c + 1) * cw], w3d[:, c * cw:(c + 1) * cw])

        idt = cpool.tile([P, P], F16, name="idt")
        make_identity(nc, idt[:])
        epst = cpool.tile([P, 1], F32, name="epst")
        nc.gpsimd.memset(epst[:], EPS)

        # ---- warmup collective: absorb cold ncfw/collective cost early ---
        wdum = cpool.tile([NCORES, NCORES], BF, name="wdum")
        nc.vector.memset(wdum[:], 0.0)
        dum_in = dpool.tile([NCORES, NCORES], BF, name="dum_in")
        nc.scalar.dma_start(dum_in[:], wdum[:])
        dum_out = dpool.tile([1, NCORES], BF, name="dum_out")
        nc.gpsimd.collective_compute(
            "ReduceScatter", OP.add, replica_groups=[list(range(NCORES))],
            ins=[dum_in.opt()], outs=[dum_out.opt()])

        hr = apool.tile([P, NT * B], BF, name="hr")
        h1n = apool.tile([P, NT * B], BF, name="h1n")
        h2n = apool.tile([P, NT * B], BF, name="h2n")

        # layer-3 psums, accumulated across all NT tiles (interleaved w/ phase B)
        psZ = [[psZp.tile([P, 512], F32, name=f"psZ{bh}{th}", tag=f"psZ{bh}{th}")
                for th in range(2)] for bh in range(2)]

        def emit_l3(t):
            for bh in range(2):
                for th in range(2):
                    nc.tensor.matmul(
                        psZ[bh][th][:],
                        lhsT=h2n[:, t * B + bh * P: t * B + (bh + 1) * P],
                        rhs=w3s[:, t * N_TF + th * 512: t * N_TF + (th + 1) * 512],
                        start=(t == 0), stop=(t == NT - 1), skip_group_check=True)

        def norm_params(st, g0, istd, nm, scale_only, gb=GB):
            """bn_stats 6-tuples (even/odd halves) -> istd (and -mean*istd)."""
            sv = st[:, g0 * 6:(g0 + gb) * 6].rearrange("p (t s) -> p t s", s=6)
            me, mo = sv[:, :, 1], sv[:, :, 4]
            M2e, M2o = sv[:, :, 2], sv[:, :, 5]
            dm = spool.tile([P, gb], F32, name="dm", tag="dm")
            nc.vector.scalar_tensor_tensor(dm[:], in0=me, scalar=1.0, in1=mo,
                                           op0=OP.mult, op1=OP.subtract)
            vq = spool.tile([P, gb], F32, name="vq", tag="vq")
            nc.vector.scalar_tensor_tensor(vq[:], in0=dm[:], scalar=0.25, in1=dm[:],
                                           op0=OP.mult, op1=OP.mult)
            var = spool.tile([P, gb], F32, name="var", tag="var")
            nc.vector.scalar_tensor_tensor(var[:], in0=M2e[:, :], scalar=1.0,
                                           in1=M2o[:, :], op0=OP.mult, op1=OP.add)
            nc.vector.tensor_scalar(out=var[:], in0=var[:], scalar1=1.0 / B,
                                    scalar2=None, op0=OP.mult)
            nc.vector.tensor_tensor(var[:], var[:], vq[:], op=OP.add)
            std = spool.tile([P, gb], F32, name="std", tag="std")
            nc.scalar.activation(std[:], var[:, :gb] if False else var[:], AF.Sqrt, bias=epst[:, 0:1])
            nc.vector.reciprocal(istd[:, :gb], std[:])
            if not scale_only:
                ms = spool.tile([P, gb], F32, name="ms", tag="ms")
                nc.vector.scalar_tensor_tensor(ms[:], in0=me, scalar=1.0, in1=mo,
                                               op0=OP.mult, op1=OP.add)
                nc.vector.scalar_tensor_tensor(nm[:, :gb], in0=ms[:], scalar=-0.5,
                                               in1=istd[:, :gb], op0=OP.mult, op1=OP.mult)

        def phase(lhs_s, rhs_get, btile, dst, hrbuf, scale_only, tail_hook=None,
                  act_scale=None, gb=GB):
            st = spool.tile([P, NT * 6], F32, name="st", tag="st")
            for g0 in range(0, NT, gb):
                for t in range(g0, g0 + gb):
                    hrt = hrbuf[:, t * B:(t + 1) * B]
                    if act_scale is not None:
                        nc.scalar.activation(hrt, rhs_get(t), AF.Relu,
                                             bias=btile[:, t:t + 1],
                                             scale=act_scale[:, t:t + 1])
                    else:
                        ps = psAB.tile([P, B], F32, name="psL", tag="psL")
                        nc.tensor.matmul(ps[:], lhsT=lhs_s[:, t * P:(t + 1) * P],
                                         rhs=rhs_get(t), start=True, stop=True)
                        nc.scalar.activation(hrt, ps[:], AF.Relu,
                                             bias=btile[:, t:t + 1])
                    nc.vector.bn_stats(st[:, t * 6:(t + 1) * 6], hrt)
                istd = spool.tile([P, GB], F32, name="istd", tag="istd")
                nm = spool.tile([P, GB], F32, name="nm", tag="nm")
                norm_params(st, g0, istd, nm, scale_only, gb)
                for t in range(g0, g0 + gb):
                    if scale_only:
                        nc.vector.tensor_scalar(out=dst[:, t * B:(t + 1) * B],
                                                in0=hrbuf[:, t * B:(t + 1) * B],
                                                scalar1=istd[:, t - g0:t - g0 + 1],
                                                scalar2=None, op0=OP.mult)
                    else:
                        nc.gpsimd.tensor_scalar(out=dst[:, t * B:(t + 1) * B],
                                                in0=hrbuf[:, t * B:(t + 1) * B],
                                                scalar1=istd[:, t - g0:t - g0 + 1],
                                                scalar2=nm[:, t - g0:t - g0 + 1],
                                                op0=OP.mult, op1=OP.add)
                    if tail_hook is not None:
                        tail_hook(t)

        phase(None, lambda t: xrep[:, t * B:(t + 1) * B], b1t, h1n, hr,
              scale_only=False, act_scale=w1t)
        # phase B: h2 is scale-only (mean shift cancels in BN3); layer-3 matmuls
        # for tile t are emitted right after h2 tile t is ready.
        phase(w2s, lambda t: h1n[:, t * B:(t + 1) * B], b2t, h2n, hr,
              scale_only=True, tail_hook=emit_l3, gb=8)

        # ---- drain z, transpose to z^T, AllToAll, reduce, BN3 ------------
        zpart = apool.tile([P, 2 * N_TF], F16, name="zpart")
        for bh in range(2):
            for th in range(2):
                nc.vector.tensor_copy(
                    zpart[:, bh * N_TF + th * 512: bh * N_TF + (th + 1) * 512],
                    psZ[bh][th][:])

        zinT = dpool.tile([N_TF, B], F16, name="zinT")
        for q in range(2):  # two DMA batches of 4 transposed tiles each
            zT4 = ztpool.tile([P, 4 * B], F16, name="zT4", tag="zT4")
            for k in range(4):
                tt = q * 4 + k
                for bh in range(2):
                    pst = psTp.tile([P, P], F16, name="pst", tag="pst")
                    nc.tensor.transpose(
                        pst[:], in_=zpart[:, bh * N_TF + tt * P: bh * N_TF + (tt + 1) * P],
                        identity=idt[:])
                    nc.vector.tensor_copy(zT4[:, k * B + bh * P: k * B + (bh + 1) * P],
                                          pst[:])
            nc.gpsimd.dma_start(
                zinT[q * 512:(q + 1) * 512, :].rearrange("(k p) b -> p k b", p=P),
                zT4[:].rearrange("p (k b) -> p k b", k=4))

        za = dpool.tile([P, B], F16, name="za")
        nc.gpsimd.collective_compute(
            "ReduceScatter", OP.add, replica_groups=[list(range(NCORES))],
            ins=[zinT.opt()], outs=[za.opt()])
        zs = ztpool.tile([P, B], F16, name="zs", tag="zs")
        nc.gpsimd.dma_start(zs[:], za[:])

        st6 = spool.tile([P, 6], F32, name="st6", tag="st6")
        nc.vector.bn_stats(st6[:], zs[:])
        mv3 = spool.tile([P, 2], F32, name="mv3", tag="mv3")
        nc.vector.bn_aggr(mv3[:], st6[:])
        std3 = spool.tile([P, 1], F32, name="std3", tag="std3")
        nc.scalar.activation(std3[:], mv3[:, 1:2], AF.Sqrt, bias=epst[:, 0:1])
        istd3 = spool.tile([P, 1], F32, name="istd3", tag="istd3")
        nc.vector.reciprocal(istd3[:], std3[:])
        nm3 = spool.tile([P, 1], F32, name="nm3", tag="nm3")
        nc.vector.scalar_tensor_tensor(nm3[:], in0=mv3[:, 0:1], scalar=-1.0,
                                       in1=istd3[:], op0=OP.mult, op1=OP.mult)
        ofin = ztpool.tile([P, B], F32, name="ofin", tag="ofin")
        nc.vector.tensor_scalar(out=ofin[:], in0=zs[:], scalar1=istd3[:],
                                scalar2=nm3[:], op0=OP.mult, op1=OP.add)
        nc.sync.dma_start(outT[:], ofin[:])

    nc.compile()
    return nc


def _pack_inputs(features, w1, b1, w2, b2, w3, b3,
                 rows1, cols1, rows2, cols2, rows3, cols3):
    """Host-side packing into per-core contiguous [128, N] tile layouts."""
    f32 = np.float32
    features = np.asarray(features, f32)
    w1 = np.asarray(w1, f32); b1 = np.asarray(b1, f32)
    w2 = np.asarray(w2, f32); b2 = np.asarray(b2, f32)
    w3 = np.asarray(w3, f32)
    rows1 = np.asarray(rows1); cols1 = np.asarray(cols1)
    rows2 = np.asarray(rows2); cols2 = np.asarray(cols2)
    rows3 = np.asarray(rows3); cols3 = np.asarray(cols3)

    w1r = np.empty(HID, f32); w1r[rows1] = w1
    c1r = np.empty(HID, np.int64); c1r[rows1] = cols1

    order2 = np.argsort(rows2, kind="stable")
    r2 = rows2[order2]; c2 = cols2[order2]; v2 = w2[order2]

    W3d = np.zeros((HID, N_TF), f32)
    np.add.at(W3d, (cols3.astype(np.int64), rows3.astype(np.int64)), w3)

    featT = np.ascontiguousarray(features.T)  # [N_GENES, B]
    in_maps = []
    for c in range(NCORES):
        hbase = c * HSH
        # xrd[p, t*B:b] = features[b, gene_of(hid row hbase + t*128 + p)]
        genes = c1r[hbase:hbase + HSH]                      # [HSH]
        xrep = featT[genes].reshape(NT, P, B).transpose(1, 0, 2).reshape(P, NT * B)

        w2t = np.zeros((NT, P, P), f32)
        for t in range(NT):
            R0 = hbase + t * P
            es = slice(WM * R0, WM * (R0 + P))
            np.add.at(w2t[t], (c2[es] - R0, r2[es] - R0), v2[es])

        w3t = W3d[hbase:hbase + HSH].reshape(NT, P, N_TF)

        in_maps.append({
            "xrd": np.ascontiguousarray(xrep).astype(BF16),
            "w1d": np.ascontiguousarray(w1r[hbase:hbase + HSH].reshape(NT, P).T),
            "w2d": np.ascontiguousarray(w2t.transpose(1, 0, 2).reshape(P, NT * P)).astype(BF16),
            "w3d": np.ascontiguousarray(w3t.transpose(1, 0, 2).reshape(P, NT * N_TF)).astype(BF16),
            "b1d": np.ascontiguousarray(b1[hbase:hbase + HSH].reshape(NT, P).T),
            "b2d": np.ascontiguousarray(b2[hbase:hbase + HSH].reshape(NT, P).T),
        })
    return in_maps


def kernel(**inputs) -> np.ndarray:
    global LAST_RESULT
    if "nc" not in _cache:
        _cache["nc"] = _build_graph()
    nc = _cache["nc"]

    in_maps = _pack_inputs(**inputs)
    # b3 is dropped: BN3 subtracts the per-TF batch mean, so a per-TF constant
    # bias cancels exactly.

    res = bass_utils.run_bass_kernel_spmd(
        nc, in_maps, core_ids=list(range(NCORES)), trace=TRACE)
    LAST_RESULT = res

    outT = np.concatenate([res.results[c]["outT"] for c in range(NCORES)], axis=0)
    return np.ascontiguousarray(outT.T.astype(np.float32))
